# revision 1
# baseline (speedup 1.0000x reference)
"""AWD-LSTM (3-layer, T=70, B=64, H=E=1024, V=32000) on 8 TRN2 NeuronCores.

Strategy:
  - Tensor-parallel over the 4H gate dimension: core k owns hidden units
    [128k, 128(k+1)) of every layer (all 4 gates for those units).
  - Feature-major [feature, batch] layout everywhere on device; the per-step
    AllGather of the 8 x [128, B] h-slices concatenates along partitions,
    directly producing the transposed h needed as matmul operand.
  - Layer-skewed scan: at slot s, cell0 processes t=s, cell1 t=s-1,
    cell2 t=s-2, so the three per-layer h exchanges fuse into ONE AllGather
    per slot and the three cells are independent within a slot.
  - px0 = Wi0 @ emb[x].T precomputed in bulk (embedding gathered by indirect
    DMA, transposed on the PE), stored in DRAM, streamed into the scan.
  - Decoder vocab-sharded: core k computes logits[:, :, 4000k:4000(k+1)],
    interleaved into scan slots as hc columns become available.
  - fp16 matmul operands (1 cy/row on PE), fp32 PSUM accumulation and fp32
    cell state / gate nonlinearities.
"""

import os
import sys

sys.path.insert(0, "/opt/trn_rl_repo")

import numpy as np

import concourse.bass as bass
import concourse.mybir as mybir
import concourse.tile as tile
from concourse import bacc
from concourse.bass_utils import run_bass_kernel_spmd

# Problem dims (hardcoded per spec)
V, E, H = 32000, 1024, 1024
T_FULL, B = 70, 64
NC = 8                 # cores
KC = H // 128          # 8 contraction chunks of 128
JS = H // NC           # 128 hidden units per core
GS = 4 * H // NC       # 512 gate rows per core
VS = V // NC           # 4000 vocab per core

DT = mybir.dt.float16
NPDT = np.float16
F32 = mybir.dt.float32
AF = mybir.ActivationFunctionType
ALU = mybir.AluOpType

_BUILD_CACHE = {}


def build(T=T_FULL, reps=1, ablate="full", dec_cap=6, split_ag=True,
          bulk_gpsimd=True):
    TOK = T * B
    NTC = TOK // 128       # 128-token chunks
    NBLK = (VS + 511) // 512

    nc = bacc.Bacc("TRN2", target_bir_lowering=False, debug=False, num_devices=NC)

    # ---- I/O ----
    emb_h = nc.dram_tensor("emb_h", [V, E], DT, kind="ExternalInput")
    xr = nc.dram_tensor("xr", [NTC, 128], mybir.dt.int32, kind="ExternalInput")
    wname = ["wi0t", "wh0t", "wi1t", "wh1t", "wi2t", "wh2t"]
    wdram = {
        n: nc.dram_tensor(n, [H, GS], DT, kind="ExternalInput") for n in wname
    }
    bdram = {
        n: nc.dram_tensor(n, [128, 4], F32, kind="ExternalInput")
        for n in ["b0", "b1", "b2"]
    }
    brdram = {
        n: nc.dram_tensor(n, [1, GS], DT, kind="ExternalInput")
        for n in ["br0", "br1", "br2"]
    }
    wdt = nc.dram_tensor("wdt", [H, VS], DT, kind="ExternalInput")
    bdd = nc.dram_tensor("bdd", [128, VS], DT, kind="ExternalInput")
    h0t = nc.dram_tensor("h0t", [3, H, B], DT, kind="ExternalInput")
    c0s = nc.dram_tensor("c0s", [3, 128, B], F32, kind="ExternalInput")
    idd = nc.dram_tensor("idd", [128, 128], DT, kind="ExternalInput")
    outd = nc.dram_tensor("outd", [TOK, VS], F32, kind="ExternalOutput")

    px0d = [
        nc.dram_tensor(f"px0d{c}", [4, 128, 128], DT) for c in range(NTC)
    ]  # internal, per 128-token chunk so the scan can start early

    with tile.TileContext(nc) as tc, tc.tile_pool(name="const", bufs=1) as const_p:
        def ptile(shape, dtype, name):
            return const_p.tile(shape, dtype, name=name, tag=name)
        # ---------- persistent SBUF ----------
        wsb = {}
        for n in wname:
            if n == "wi0t":
                continue  # scoped to phase 1 below
            w = ptile([128, KC * GS], DT, f"{n}_sb")
            nc.sync.dma_start(
                w[:].rearrange("p (kc m) -> p kc m", kc=KC),
                wdram[n][:].rearrange("(kc p) m -> p kc m", p=128),
            )
            wsb[n] = w
        bsb = {}
        for n in ["b0", "b1", "b2"]:
            b = ptile([128, 4], F32, f"{n}_sb")
            nc.sync.dma_start(b[:], bdram[n][:])
            bsb[n] = b
        brsb = {}
        for n in ["br0", "br1", "br2"]:
            br = ptile([1, GS], DT, f"{n}_sb")
            nc.sync.dma_start(br[:], brdram[n][:])
            brsb[n] = br
        ones64 = ptile([1, B], DT, "ones64")
        nc.gpsimd.memset(ones64[:], 1.0)
        wds = ptile([128, KC * VS], DT, "wds")
        nc.sync.dma_start(
            wds[:].rearrange("p (kc m) -> p kc m", kc=KC),
            wdt[:].rearrange("(kc p) m -> p kc m", p=128),
        )
        ident = ptile([128, 128], DT, "ident")
        nc.sync.dma_start(ident[:], idd[:])
        h0sb = []
        csb = []
        for l in range(3):
            h0l = ptile([128, KC * B], DT, f"h0sb{l}")
            nc.sync.dma_start(
                h0l[:].rearrange("p (kc b) -> p kc b", kc=KC),
                h0t[l].rearrange("(kc p) b -> p kc b", p=128),
            )
            h0sb.append(h0l)
            cl = ptile([128, B], F32, f"csb{l}")
            csb.append(cl)
        zrow = ptile([128, B], DT, "zrow")
        nc.gpsimd.memset(zrow[:], 0.0)
        outT = ptile([128, KC * TOK], DT, "outT")
        outT_v = outT[:].rearrange("p (kc tok) -> p kc tok", kc=KC)

        for _rep in range(reps):
          for l in range(3):
            nc.sync.dma_start(csb[l][:], c0s[l])
          # ---------- phase 1: gather embeddings, transpose, px0 ----------
          if ablate in ("full", "noag", "nodec", "noscan"):
           with (
            tc.tile_pool(name="xi_p", bufs=2) as xi_p,
            tc.tile_pool(name="xe_p", bufs=2) as xe_p,
            tc.tile_pool(name="xt_p", bufs=2) as xt_p,
            tc.tile_pool(name="wi0_p", bufs=1) as wi0_p,
            tc.tile_pool(name="tp_ps", bufs=2, space="PSUM") as tp_ps,
            tc.tile_pool(name="px_ps", bufs=2, space="PSUM") as px_ps,
            tc.tile_pool(name="pxs_p", bufs=2) as pxs_p,
          ):
            wi0s = wi0_p.tile([128, KC * GS], DT)
            nc.sync.dma_start(
                wi0s[:].rearrange("p (kc m) -> p kc m", kc=KC),
                wdram["wi0t"][:].rearrange("(kc p) m -> p kc m", p=128),
            )
            for c in range(NTC):
                xi = xi_p.tile([128, 1], mybir.dt.int32)
                nc.sync.dma_start(xi[:], xr[c : c + 1, :].rearrange("a p -> p a"))
                xe = xe_p.tile([128, E], DT)
                nc.gpsimd.indirect_dma_start(
                    out=xe[:],
                    out_offset=None,
                    in_=emb_h[:],
                    in_offset=bass.IndirectOffsetOnAxis(ap=xi[:, :1], axis=0),
                )
                xet = xt_p.tile([128, KC * 128], DT)
                for kc in range(KC):
                    tp = tp_ps.tile([128, 128], DT)
                    nc.tensor.transpose(
                        tp[:], xe[:, kc * 128 : (kc + 1) * 128], ident[:]
                    )
                    nc.vector.tensor_copy(
                        xet[:, kc * 128 : (kc + 1) * 128], tp[:]
                    )
                for g in range(4):
                    pp = px_ps.tile([128, 128], F32)
                    for kc in range(KC):
                        nc.tensor.matmul(
                            pp[:],
                            lhsT=wi0s[:].rearrange(
                                "p (kc m) -> p kc m", kc=KC
                            )[:, kc, g * 128 : (g + 1) * 128],
                            rhs=xet[:, kc * 128 : (kc + 1) * 128],
                            start=(kc == 0),
                            stop=(kc == KC - 1),
                        )
                    pxs = pxs_p.tile([128, 128], DT)
                    nc.vector.tensor_copy(pxs[:], pp[:])
                    nc.sync.dma_start(px0d[c][g, :, :], pxs[:])

          # ---------- phase 2: skewed scan + interleaved decoder ----------
          if ablate in ("full", "noag", "nodec", "nopx"):
           from contextlib import ExitStack as _ES
           with _ES() as _es:
            def _pool(name, bufs, space="SBUF"):
                return _es.enter_context(
                    tc.tile_pool(name=name, bufs=bufs, space=space))
            h_p = _pool("h_p", 2)
            px_p = _pool("px_p", 3)
            g_ps = _pool("g_ps", 3, "PSUM")
            sig_p = _pool("sig_p", 3)
            tg_p = _pool("tg_p", 3)
            tc_p = _pool("tc_p", 3)
            t1_p = _pool("t1_p", 2)
            t2_p = _pool("t2_p", 2)
            h2_p = _pool("h2_p", 3)
            agi_p = _pool("agi_p", 3, "DRAM")
            ago_p = _pool("ago_p", 3, "DRAM")
            agis0 = _pool("agis0", 3, "DRAM")
            agis1 = _pool("agis1", 3, "DRAM")
            agis2 = _pool("agis2", 3, "DRAM")
            agos0 = _pool("agos0", 3, "DRAM")
            agos1 = _pool("agos1", 3, "DRAM")
            agos2 = _pool("agos2", 3, "DRAM")
            d_ps = _pool("d_ps", 2, "PSUM")
            ds_p = _pool("ds_p", 2)
            bd_p = _pool("bd_p", 2)
            bulk_dma = nc.gpsimd.dma_start if bulk_gpsimd else nc.sync.dma_start
            wv = {n: wsb[n][:].rearrange("p (kc m) -> p kc m", kc=KC) for n in wname if n != "wi0t"}
            wdv = wds[:].rearrange("p (kc m) -> p kc m", kc=KC)

            def decoder_tile(j, vt):
                    n0 = 512 * vt
                    nn = min(512, VS - n0)
                    dp = d_ps.tile([128, 512], F32)
                    for kc in range(KC):
                        nc.tensor.matmul(
                            dp[:, :nn],
                            lhsT=outT[
                                :, kc * TOK + 128 * j : kc * TOK + 128 * j + 128
                            ],
                            rhs=wdv[:, kc, n0 : n0 + nn],
                            start=(kc == 0),
                            stop=(kc == KC - 1),
                        )
                    bdt = bd_p.tile([128, 512], DT)
                    bulk_dma(bdt[:, :nn], bdd[:, n0 : n0 + nn])
                    ds = ds_p.tile([128, 512], F32)
                    nc.vector.tensor_tensor(
                        out=ds[:, :nn],
                        in0=dp[:, :nn],
                        in1=bdt[:, :nn],
                        op=ALU.add,
                    )
                    bulk_dma(
                        outd[128 * j : 128 * j + 128, n0 : n0 + nn], ds[:, :nn]
                    )

            agis_p = [agis0, agis1, agis2]
            agos_p = [agos0, agos1, agos2]
            ago3_cur = [None, None, None]
            ago_prev = None
            dec_q = []
            next_blk = 0
            NSLOT = T + 2
            for s in range(NSLOT):
                # h state tiles for this slot (full transposed h per layer)
                hcur = []
                for l in range(3):
                    if s == 0:
                        hcur.append(None)  # cells use h0sb at t==0
                        continue
                    if split_ag and ago_prev[l] is None:
                        hcur.append(None)
                        continue
                    hl = h_p.tile([128, KC * B], DT, name=f"hcur{l}", tag=f"hcur{l}")
                    src = (ago_prev[l][:, :, :] if split_ag
                           else ago_prev[:, l, :, :])
                    nc.sync.dma_start(
                        hl[:].rearrange("p (c b) -> p c b", c=NC),
                        src.rearrange("c p b -> p c b"),
                    )
                    hcur.append(hl)
                if s >= 1:
                    # accumulate decoder input: full hc for t = s - 3
                    t2_ = s - 3
                    if 0 <= t2_ < T:
                        src2 = (ago_prev[2][:, :, :] if split_ag
                                else ago_prev[:, 2, :, :])
                        nc.sync.dma_start(
                            outT_v[:, :, 64 * t2_ : 64 * t2_ + 64],
                            src2.rearrange("c p b -> p c b"),
                        )

                if split_ag:
                    agin3 = [agis_p[l].tile([128, B], DT, name=f"agi{l}",
                                            tag=f"agi{l}") for l in range(3)]
                else:
                    agin = agi_p.tile([3, 128, B], DT)
                for l in range(3):
                    t_l = s - l
                    if not (0 <= t_l < T):
                        if not split_ag:
                            nc.sync.dma_start(agin[l, :, :], zrow[:])
                        continue
                    wi = wv[wname[2 * l]] if l > 0 else None
                    wh = wv[wname[2 * l + 1]]
                    hrec = h0sb[l][:] if t_l == 0 else hcur[l][:]
                    psum = g_ps.tile([128, 256], F32, name=f"g{l}", tag="gps")
                    if l == 0:
                        px = px_p.tile([128, 256], DT)
                        o64 = (t_l % 2) * 64
                        nc.sync.dma_start(
                            px[:].rearrange("p (g b) -> p g b", g=4),
                            px0d[t_l // 2][:, :, o64 : o64 + 64].rearrange(
                                "g p b -> p g b"
                            ),
                        )
                    hin = None if l == 0 else (
                        h0sb[l - 1][:] if s == 0 else hcur[l - 1][:]
                    )
                    for g in range(4):
                        gs = psum[:, 64 * g : 64 * g + 64]
                        if l == 0:
                            nc.tensor.matmul(
                                gs, lhsT=ident[:],
                                rhs=px[:, 64 * g : 64 * g + 64],
                                start=True, stop=False,
                            )
                        else:
                            for kc in range(KC):
                                nc.tensor.matmul(
                                    gs,
                                    lhsT=wi[:, kc, g * 128 : (g + 1) * 128],
                                    rhs=hin[:, kc * B : kc * B + B],
                                    start=(kc == 0), stop=False,
                                )
                        for kc in range(KC):
                            nc.tensor.matmul(
                                gs,
                                lhsT=wh[:, kc, g * 128 : (g + 1) * 128],
                                rhs=hrec[:, kc * B : kc * B + B],
                                start=False, stop=False,
                            )
                        nc.tensor.matmul(
                            gs,
                            lhsT=brsb[f"br{l}"][0:1, g * 128 : (g + 1) * 128],
                            rhs=ones64[0:1, :],
                            start=False, stop=True,
                        )
                    sig = sig_p.tile([128, 192], F32, name=f"sig{l}", tag="sig")
                    nc.scalar.activation(sig[:], psum[:, 0:192], AF.Sigmoid)
                    tg = tg_p.tile([128, B], F32, name=f"tg{l}", tag="tg")
                    nc.scalar.activation(tg[:], psum[:, 192:256], AF.Tanh)
                    t1 = t1_p.tile([128, B], F32, name=f"t1{l}", tag="t1")
                    t2 = t2_p.tile([128, B], F32, name=f"t2{l}", tag="t2")
                    nc.vector.tensor_tensor(
                        out=t1[:], in0=sig[:, 64:128], in1=csb[l][:], op=ALU.mult
                    )
                    nc.vector.tensor_tensor(
                        out=t2[:], in0=sig[:, 0:64], in1=tg[:], op=ALU.mult
                    )
                    nc.vector.tensor_tensor(
                        out=csb[l][:], in0=t1[:], in1=t2[:], op=ALU.add
                    )
                    tch = tc_p.tile([128, B], F32, name=f"tc{l}", tag="tc")
                    nc.scalar.activation(tch[:], csb[l][:], AF.Tanh)
                    h2 = h2_p.tile([128, B], DT, name=f"h2{l}", tag="h2")
                    nc.vector.tensor_tensor(
                        out=h2[:], in0=sig[:, 128:192], in1=tch[:], op=ALU.mult
                    )
                    if split_ag:
                        nc.sync.dma_start(agin3[l][:], h2[:])
                        ago3l = agos_p[l].tile([NC, 128, B], DT, name=f"ago{l}",
                                               tag=f"ago{l}")
                        if ablate != "noag":
                            nc.gpsimd.collective_compute(
                                "AllGather", ALU.bypass,
                                replica_groups=[list(range(NC))],
                                ins=[agin3[l][:].opt()],
                                outs=[ago3l[:].opt()],
                            )
                        ago3_cur[l] = ago3l
                    else:
                        nc.sync.dma_start(agin[l, :, :], h2[:])

                if split_ag:
                    ago_prev = list(ago3_cur)
                else:
                    ago = ago_p.tile([NC, 3, 128, B], DT)
                    if ablate != "noag":
                        nc.gpsimd.collective_compute(
                            "AllGather",
                            ALU.bypass,
                            replica_groups=[list(range(NC))],
                            ins=[agin[:].opt()],
                            outs=[ago[:].opt()],
                        )
                    ago_prev = ago

                # interleave decoder work at vocab-tile granularity:
                # block j needs hc for t=2j+1, in outT after slot (2j+1)+3
                if ablate != "nodec":
                    while next_blk < NTC and 2 * next_blk + 4 <= s:
                        dec_q.extend((next_blk, vt) for vt in range(NBLK))
                        next_blk += 1
                    for _ in range(min(dec_cap, len(dec_q))):
                        decoder_tile(*dec_q.pop(0))

            # final hc (t = T-1) sits in the last slot's AllGather output
            srcf = (ago_prev[2][:, :, :] if split_ag else ago_prev[:, 2, :, :])
            nc.sync.dma_start(
                outT_v[:, :, 64 * (T - 1) : 64 * (T - 1) + 64],
                srcf.rearrange("c p b -> p c b"),
            )
            if ablate != "nodec":
                while next_blk < NTC:
                    dec_q.extend((next_blk, vt) for vt in range(NBLK))
                    next_blk += 1
                for jvt in dec_q:
                    decoder_tile(*jvt)

    nc.compile()
    return nc


def _prep_inputs(x, h0, c0, emb, Wi0, bi0, Wh0, bh0, Wi1, bi1, Wh1, bh1,
                 Wi2, bi2, Wh2, bh2, Wd, bd, T):
    """Shard + lay out inputs for the 8 cores."""
    TOK = T * B
    NTC = TOK // 128
    x = np.asarray(x)[:T]
    xr = np.ascontiguousarray(
        x.reshape(-1).astype(np.int32).reshape(NTC, 128)
    )
    emb_h = np.asarray(emb, dtype=NPDT)
    h0t = np.ascontiguousarray(
        np.asarray(h0, dtype=NPDT).transpose(0, 2, 1)
    )  # [3, H, B]
    ident = np.eye(128, dtype=NPDT)

    Ws = {
        "wi0t": np.asarray(Wi0), "wh0t": np.asarray(Wh0),
        "wi1t": np.asarray(Wi1), "wh1t": np.asarray(Wh1),
        "wi2t": np.asarray(Wi2), "wh2t": np.asarray(Wh2),
    }
    bsum = {
        0: np.asarray(bi0) + np.asarray(bh0),
        1: np.asarray(bi1) + np.asarray(bh1),
        2: np.asarray(bi2) + np.asarray(bh2),
    }
    Wd = np.asarray(Wd)
    bd = np.asarray(bd)
    c0 = np.asarray(c0)

    in_maps = []
    for k in range(NC):
        rows = np.concatenate(
            [np.arange(1024 * q + 128 * k, 1024 * q + 128 * (k + 1))
             for q in range(4)]
        )
        m = {"emb_h": emb_h, "xr": xr, "h0t": h0t, "idd": ident}
        for n, W in Ws.items():
            m[n] = np.ascontiguousarray(W[rows, :].T.astype(NPDT))
        for l in range(3):
            m[f"b{l}"] = np.ascontiguousarray(
                bsum[l][rows].reshape(4, 128).T.astype(np.float32)
            )
            m[f"br{l}"] = np.ascontiguousarray(
                bsum[l][rows].reshape(1, -1).astype(NPDT)
            )
        m["wdt"] = np.ascontiguousarray(
            Wd[VS * k : VS * (k + 1), :].T.astype(NPDT)
        )
        m["bdd"] = np.ascontiguousarray(
            np.broadcast_to(bd[VS * k : VS * (k + 1)], (128, VS)).astype(NPDT)
        )
        m["c0s"] = np.ascontiguousarray(
            c0[:, :, 128 * k : 128 * (k + 1)].transpose(0, 2, 1)
        ).astype(np.float32)
        in_maps.append(m)
    return in_maps


def kernel(x, h0, c0, emb, Wi0, bi0, Wh0, bh0, Wi1, bi1, Wh1, bh1,
           Wi2, bi2, Wh2, bh2, Wd, bd, _T=None, _trace=False):
    T = _T or T_FULL
    if T not in _BUILD_CACHE:
        _BUILD_CACHE[T] = build(T)
    nc = _BUILD_CACHE[T]
    in_maps = _prep_inputs(
        x, h0, c0, emb, Wi0, bi0, Wh0, bh0, Wi1, bi1, Wh1, bh1,
        Wi2, bi2, Wh2, bh2, Wd, bd, T,
    )
    res = run_bass_kernel_spmd(
        nc, in_maps, core_ids=list(range(NC)), trace=_trace
    )
    kernel.last_result = res
    out = np.concatenate(
        [res.results[k]["outd"] for k in range(NC)], axis=1
    )
    return out.reshape(T, B, V)

